# revision 8
# baseline (speedup 1.0000x reference)
"""Bhattacharyya coefficient kernel for Trainium2 (8 NeuronCores, SPMD).

out[n,0,i,j] = (1/k^2) * sum_{c,p,q} w[c] * sqrt(x[n,c,i+p,j+q] * z[n,c,p,q])

Data-parallel over batch: 2 samples per core. Per sample:
  1. ACT: sx = sqrt(x) (bf16), szw = w/k^2 * sqrt(z) (bf16).
  2. TensorE: plane[t, y] = sum_c szw[c, t] * sx[c, y] for the 64 taps
     t = 8p+q and all 63*63 pixels y (K=256 as two 128-chunks in PSUM).
  3. DVE: evict PSUM into fp8-e4m3 plane pieces matching the DRAM
     scratch tensors exactly (one dump DMA each, precise deps).
  4. Dump -> DRAM scratch and gather back tap-aligned via a flat-DRAM AP
     A[t, u] = plane[t, u + 63*(t>>3) + (t&7)]  (both on the SWDGE ring:
     the Sync ring is FIFO and anything behind the 8.1MB x stream would
     wait for all of it).
  5. TensorE ones-matmul o[u] = sum_t A[t, u] (fp8), DVE evicts psum2 to
     obuf, out[i, j] = o[63 i + j] shipped by row-range DMAs (Sync; by
     then the x stream has drained).

Engines execute in (roughly) emission order, so stage-2 *compute* for a
sample is deliberately emitted interleaved into the NEXT sample's
stage-1 at matching readiness; otherwise the next sample's evictions
queue behind obuf copies that wait on the dump/gather DMA chain
(measured +13us).  x is staged in 2-block (512KB) pieces so the first
PSUM blocks close early and stage-2 overlaps the stream.

fp8 plane: values are O(1)..4 and the tap sum averages 64 independent
quantization errors -> absmax rel err ~1.1e-2 < the 2e-2 gate, while
halving scratch DRAM traffic.
"""

import numpy as np

import concourse.bacc as bacc
import concourse.bass as bass
import concourse.mybir as mybir
from concourse import tile
from concourse.bass_utils import run_bass_kernel_spmd

N, C, KS, MS = 16, 256, 8, 63
MO = MS - KS + 1            # 56
F = MS * MS                 # 3969
NCORES = 8
SPC = N // NCORES           # samples per core
BLK = 512
W = (MO - 1) * MS + MO      # 3521: last needed o index is 63*55+55
SH = 448                    # max tap shift: 63*7 + 7
AF = mybir.ActivationFunctionType
f32 = mybir.dt.float32
bf16 = mybir.dt.bfloat16
fp8 = mybir.dt.float8e4

# x staging pieces (start block, n blocks)
PIECES = [(0, 2), (2, 2), (4, 2), (6, 2)]

# gather chunks (u0, u1) in o-index space, split at output-row
# boundaries (row i covers u = 63 i .. 63 i + 55); each <= 1008 so
# stage-2 fits one [1, 1024] 2-bank PSUM tile.
GCH = [(0, 1008), (1008, 2016), (2016, 3024), (3024, W)]
SCR = [(0, 2016 + SH), (2016, 3024 + SH), (3024, F)]
CHUNK_SC = [0, 0, 1, 2]
SC_LASTBLK = [(c1 - 1) // BLK for (_, c1) in SCR]   # [4, 6, 7]
# output rows: chunks 0+1 -> rows [0,32), chunks 2+3 -> rows [32,56)
OUT_ROWS = {1: (0, 32), 3: (32, MO)}

_CACHE = {}


def _build():
    nc = bacc.Bacc("TRN2", target_bir_lowering=False, debug=False)
    z_in = nc.declare_dram_parameter("z", [SPC, C, KS, KS], f32, isOutput=False)
    x_in = nc.declare_dram_parameter("x", [SPC, C, MS, MS], f32, isOutput=False)
    w_in = nc.declare_dram_parameter("w", [C], f32, isOutput=False)
    out = nc.declare_dram_parameter("out", [SPC, 1, MO, MO], f32, isOutput=True)

    scs = [
        [nc.dram_tensor(f"sc{ci}_{s}", [64, c1 - c0], fp8)
         for ci, (c0, c1) in enumerate(SCR)]
        for s in range(SPC)
    ]

    xflat = x_in.rearrange("s (k c) h w -> s k c (h w)", c=128)  # [SPC,2,128,F]

    with tile.TileContext(nc) as tc:
        with (
            tc.tile_pool(name="xstage", bufs=16) as xstage,
            tc.tile_pool(name="sxq", bufs=6) as sxq,
            tc.tile_pool(name="zpool", bufs=2) as zpool,
            tc.tile_pool(name="plane", bufs=2) as plane,
            tc.tile_pool(name="gath", bufs=4) as gath,
            tc.tile_pool(name="opool", bufs=1) as opool,
            tc.tile_pool(name="psum", bufs=4, space="PSUM") as psum,
            tc.tile_pool(name="psum2", bufs=2, space="PSUM") as psum2,
        ):
            # ---- Sync ring: first x piece, then the tiny z/w loads
            # (complete by the time the z path needs them), then the rest
            # of the x stream.  Nothing else rides this queue until the
            # final out DMAs.
            xst = {}

            def load_piece(s, k, pi):
                b0, nbk = PIECES[pi]
                lo = b0 * BLK
                ln = min(nbk * BLK, F - lo)
                t = xstage.tile([128, 2 * BLK], f32, tag="xst",
                                name=f"xst{s}{k}{pi}")
                nc.sync.dma_start(t[:, :ln], xflat[s, k, :, lo : lo + ln])
                xst[(s, k, pi)] = t

            load_piece(0, 0, 0)
            wt = zpool.tile([128, 2], f32, name="wt")
            nc.sync.dma_start(wt[:], w_in.rearrange("(k c) -> c k", c=128))
            zts = []
            for s in range(SPC):
                zt = zpool.tile([128, 2, KS * KS], f32, tag="zt", name=f"zt{s}")
                nc.sync.dma_start(
                    zt[:], z_in[s].rearrange("(k c) p q -> c k (p q)", c=128)
                )
                zts.append(zt)
            load_piece(0, 1, 0)
            for s in range(SPC):
                for pi in range(len(PIECES)):
                    for k in range(2):
                        if (s, k, pi) != (0, 0, 0) and (s, k, pi) != (0, 1, 0):
                            load_piece(s, k, pi)

            ones = opool.tile([64, 1], fp8, name="ones")
            nc.gpsimd.memset(ones[:], 1.0)
            w64 = zpool.tile([128, 2], f32, name="w64")
            nc.vector.tensor_scalar_mul(w64[:], wt[:], 1.0 / (KS * KS))

            obufs, szws, planes, ats = {}, {}, {}, {}

            def emit_sc_dma(s, ci):
                """dump scratch tensor ci and issue the gathers of its
                chunks (all SWDGE; runs as soon as data is ready)."""
                c0, c1 = SCR[ci]
                pit = c1 - c0
                nc.gpsimd.dma_start(scs[s][ci][:, :], planes[s][ci][:])
                for ch, (u0, u1) in enumerate(GCH):
                    if CHUNK_SC[ch] != ci:
                        continue
                    ulen = u1 - u0
                    a = gath.tile([64, 1008], fp8, tag="a", name=f"a{s}_{ch}")
                    src = bass.AP(
                        scs[s][ci][:].tensor,
                        u0 - c0,
                        [[8 * pit + MS, 8], [pit + 1, 8], [1, ulen]],
                    )
                    nc.gpsimd.dma_start(a[:, :ulen], src)
                    ats[(s, ch)] = a

            def emit_compute(s, ch):
                """tap-reduce matmuls for chunk ch + obuf eviction (+ out
                DMA when the chunk completes an output-row range)."""
                u0, u1 = GCH[ch]
                ulen = u1 - u0
                a = ats[(s, ch)]
                ps2 = psum2.tile([1, 2 * BLK], f32, tag="ps2",
                                 name=f"ps2_{s}_{ch}")
                for m0 in range(0, ulen, BLK):
                    nb = min(BLK, ulen - m0)
                    nc.tensor.matmul(
                        ps2[:, m0 : m0 + nb],
                        ones[:],
                        a[:, m0 : m0 + nb],
                        start=True,
                        stop=True,
                    )
                ob = obufs[s][ch // 2]
                base = 0 if ch < 2 else GCH[2][0]
                nc.vector.tensor_copy(
                    ob[0:1, u0 - base : u1 - base], ps2[:, :ulen]
                )
                if ch in OUT_ROWS:
                    r0, r1 = OUT_ROWS[ch]
                    nr = r1 - r0
                    osrc = ob[0:1, 0 : nr * MS].rearrange(
                        "p (i j) -> p i j", i=nr
                    )[:, :, 0:MO]
                    nc.sync.dma_start(out[s, 0, r0:r1].unsqueeze(0), osrc)

            # stage-2 compute hooks: (sample, end-of-piece) -> chunks.
            # A sample's late chunks are woven into the NEXT sample's
            # stage-1 so engine program order matches data readiness.
            hooks = {}
            for s in range(SPC):
                hooks[(s, 2)] = [(s, 0), (s, 1)]
                if s + 1 < SPC:
                    hooks[(s + 1, 0)] = [(s, 2)]
                    hooks[(s + 1, 1)] = [(s, 3)]
                else:
                    hooks[(s, 3)] = [(s, 2), (s, 3)]

            for s in range(SPC):
                obufs[s] = {
                    0: opool.tile([1, 2016], f32, tag=f"obA{s}",
                                  name=f"obufA{s}"),
                    1: opool.tile([1, 1536], f32, tag=f"obB{s}",
                                  name=f"obufB{s}"),
                }
                zsq = zpool.tile([128, 2, KS * KS], f32, tag="zsq",
                                 name=f"zsq{s}")
                szw = zpool.tile([128, 2, KS * KS], bf16, tag="szw",
                                 name=f"szw{s}")
                for kk in range(2):
                    nc.scalar.activation(zsq[:, kk, :], zts[s][:, kk, :],
                                         AF.Sqrt)
                    nc.vector.tensor_scalar_mul(
                        szw[:, kk, :], zsq[:, kk, :], w64[:, kk : kk + 1]
                    )
                szws[s] = szw
                planes[s] = [
                    plane.tile([64, c1 - c0], fp8, tag=f"pl{ci}",
                               name=f"pl{s}_{ci}")
                    for ci, (c0, c1) in enumerate(SCR)
                ]
                evmap = [[] for _ in range(8)]
                for ci, (c0, c1) in enumerate(SCR):
                    for b in range(8):
                        lo = max(c0, b * BLK)
                        hi = min(c1, (b + 1) * BLK, F)
                        if lo < hi:
                            evmap[b].append((ci, lo - b * BLK, hi - b * BLK,
                                             lo - c0))

                for pi, (b0, nbk) in enumerate(PIECES):
                    lo = b0 * BLK
                    ln = min(nbk * BLK, F - lo)
                    sxp = {}
                    for k in range(2):
                        t = sxq.tile([128, 2 * BLK], bf16, tag="sxp",
                                     name=f"sxp{s}{k}{pi}")
                        nc.scalar.activation(
                            t[:, :ln], xst[(s, k, pi)][:, :ln], AF.Sqrt
                        )
                        sxp[k] = t
                    for j in range(nbk):
                        b = b0 + j
                        nb = min(BLK, F - b * BLK)
                        ps = psum.tile([64, BLK], f32, tag="ps",
                                       name=f"ps_{s}_{b}")
                        for k in range(2):
                            nc.tensor.matmul(
                                ps[:, :nb],
                                szw[:, k, :],
                                sxp[k][:, j * BLK : j * BLK + nb],
                                start=(k == 0),
                                stop=(k == 1),
                            )
                        for (ci, p_lo, p_hi, d_lo) in evmap[b]:
                            nc.vector.tensor_copy(
                                planes[s][ci][:, d_lo : d_lo + (p_hi - p_lo)],
                                ps[:, p_lo:p_hi],
                            )
                        for ci in range(len(SCR)):
                            if SC_LASTBLK[ci] == b:
                                emit_sc_dma(s, ci)
                    for item in hooks.get((s, pi), ()):
                        emit_compute(*item)

    nc.compile()
    return nc


def _get_nc():
    if "nc" not in _CACHE:
        _CACHE["nc"] = _build()
    return _CACHE["nc"]


def _run(z, x, weights, **runkw):
    z = np.ascontiguousarray(np.asarray(z), dtype=np.float32)
    x = np.ascontiguousarray(np.asarray(x), dtype=np.float32)
    w = np.ascontiguousarray(np.asarray(weights), dtype=np.float32).reshape(C)
    in_maps = []
    for i in range(NCORES):
        lo, hi = i * SPC, (i + 1) * SPC
        in_maps.append({"z": z[lo:hi], "x": x[lo:hi], "w": w})
    nc = _get_nc()
    try:
        res = run_bass_kernel_spmd(
            nc, in_maps, core_ids=list(range(NCORES)), **runkw
        )
    except Exception:
        # transient device errors (e.g. NRT exec-unit unrecoverable) have
        # been observed to succeed on retry
        res = run_bass_kernel_spmd(
            nc, in_maps, core_ids=list(range(NCORES)), **runkw
        )
    full = np.concatenate([res.results[i]["out"] for i in range(NCORES)], axis=0)
    return full, res


def kernel(z, x, weights):
    full, _ = _run(z, x, weights)
    return full
